# revision 10
# baseline (speedup 1.0000x reference)
"""DenseVariational bass kernel for TRN2 (8 NeuronCores).

Problem: out[s,b,o] = sum_i input[s,b,i] * (mu[o,i] + softplus(rho[o,i])*eps_w[s,o,i])
                      + bias_mu[o] + softplus(bias_rho[o])*eps_b[s,o]
  S=32 samples, B=256, IN=OUT=1024, fp32 inputs.

Sharding: samples split 4-per-core across 8 cores; mu/rho replicated.

Per-core device program (mixed precision, DMA-bound design):
  - All large operands are downcast to fp16 ON INGEST via gpsimd (SWDGE)
    casting DMAs: rho/mu/x/eps stream in as fp16, halving HBM-side DMA cost.
    fp16 keeps 10 mantissa bits -> ~6e-4 relative output error, far inside
    the 2e-2 gate, while fp16 matmuls run at 1 PE cycle/row (4x fp32).
  - rho arrives in single-k-tile chunks so ScalarE softplus (Exp, Ln(1+x))
    starts ~3us in; sigma gates the first sample's DVE multiplies.
  - per sample: eps.T in 2-k-tile chunks; DVE computes W.T = sigma.T*eps.T
    (+ mu.T for samples 0-2) in place (fp16 2x mode); PE accumulates
    psum[ob] += W.T[kt,ob].T @ X.T[kt].  For the LAST sample the mu add is
    folded into PE instead (psum += mu.T[kt,ob].T @ X.T[kt]) to balance
    DVE-end against PE-end at the tail.
  - bias[s,o] enters PSUM via rank-1 matmuls (bias row stationary, ones
    moving).  The bias row is computed in a fast partition-parallel layout
    and moved to a single-partition row by a tiny SBUF->SBUF DMA.
  - PSUM -> SBUF stage (fp16) via plain [P,512] Identity copies on ScalarE;
    the last sample spreads its four copies over ScalarE/DVE/Pool so the
    tail drains in parallel; stage written to DRAM as fp16 (host upcasts).
  - PE gets NO work until ~9us; the p-state model then starts the Tensor
    engine at full clock (never-busy => fully ramped) and the stream keeps
    it busy to the end.
  - Tile pools are deep enough that loads never wait on compute.

DMA queues: gpsimd (casting loads - the serialized-bandwidth bottleneck),
sync (tiny fp32 bias loads + bias-row shuffle), scalar (fp16 stores).

Host pre-arranges layouts (pure data movement, part of sharding; no host
arithmetic - dtype casts happen on device):
  xt[s][p, kt*256+b]  = input[s, b, kt*128+p]         (fp32)
  epst[s][i, o]       = eps_w[s, o, i]                (fp32)
  mut/rhot[i, o]      = mu/rho[o, i]                  (fp32)
  epsb_so[s*8+ob, p]  = eps_b[s, ob*128+p]            (fp32)
  bmu_so/brho_so[s*8+ob, p] = bias_mu/bias_rho[ob*128+p]  (fp32, replicated)
  output yt[s][p, ob*256+b] = out[s, b, ob*128+p]     (fp16; host upcasts)
"""

import numpy as np

import concourse.bass as bass
import concourse.mybir as mybir
import concourse.tile as tile
from concourse import bacc
from concourse.bass_utils import run_bass_kernel_spmd

# Problem constants (hardcoded per harness contract)
S, B, IN, OUT = 32, 256, 1024, 1024
NCORES = 8
SL = S // NCORES          # samples per core = 4
P = 128
KT = IN // P              # 8 k-tiles
OB = OUT // P             # 8 output-row blocks
FP32 = mybir.dt.float32
FP16 = mybir.dt.float16
ActF = mybir.ActivationFunctionType

# eps chunk sizes (k-tiles) per sample; last sample tapers for a short tail
CHUNKS = [[2, 2, 2, 2], [2, 2, 2, 2], [2, 2, 2, 2], [2, 2, 2, 1, 1]]
MU_ON_PE = {SL - 1}       # samples whose mu-add runs as PE matmuls

_cached = None


def build_bass():
    nc = bacc.Bacc(
        "TRN2",
        target_bir_lowering=False,
        debug=False,
        enable_asserts=False,
        num_devices=NCORES,
    )

    xt = nc.dram_tensor("xt", (SL, P, KT * B), FP32, kind="ExternalInput").ap()
    epst = nc.dram_tensor("epst", (SL, IN, OUT), FP32, kind="ExternalInput").ap()
    mut = nc.dram_tensor("mut", (IN, OUT), FP32, kind="ExternalInput").ap()
    rhot = nc.dram_tensor("rhot", (IN, OUT), FP32, kind="ExternalInput").ap()
    bmu_so = nc.dram_tensor("bmu_so", (SL * OB, P), FP32, kind="ExternalInput").ap()
    brho_so = nc.dram_tensor("brho_so", (SL * OB, P), FP32, kind="ExternalInput").ap()
    epsb_so = nc.dram_tensor("epsb_so", (SL * OB, P), FP32, kind="ExternalInput").ap()
    bias_scr = nc.dram_tensor("bias_scr", (SL * OB, P), FP16, kind="Internal").ap()
    yt = nc.dram_tensor("yt", (SL, P, OB * B), FP16, kind="ExternalOutput").ap()

    mut_r = mut.rearrange("(kt p) o -> p kt o", p=P)
    rhot_r = rhot.rearrange("(kt p) o -> p kt o", p=P)

    with tile.TileContext(nc) as tc:
        with (
            tc.tile_pool(name="persist", bufs=1) as persist,
            tc.tile_pool(name="eps", bufs=12) as eps_pool,
            tc.tile_pool(name="xtp", bufs=4) as xt_pool,
            tc.tile_pool(name="outp", bufs=2) as out_pool,
            tc.tile_pool(name="psum", bufs=2, space="PSUM") as psum_pool,
        ):
            mu_sb = persist.tile([P, KT, OUT], FP16)
            sig_sb = persist.tile([P, KT, OUT], FP16)
            sigb = persist.tile([SL * OB, P], FP32)
            bmu_sb = persist.tile([SL * OB, P], FP32)
            bias32 = persist.tile([SL * OB, P], FP32)
            bias16_2d = persist.tile([SL * OB, P], FP16)
            bias_row = persist.tile([1, SL * OB * P], FP16)
            ones = persist.tile([1, B], FP16)

            # tiny fp32 bias loads on the sync queue (HWDGE, non-cast)
            nc.sync.dma_start(out=sigb[:], in_=brho_so[:])
            nc.sync.dma_start(out=bmu_sb[:], in_=bmu_so[:])
            nc.sync.dma_start(out=bias32[:], in_=epsb_so[:])
            nc.vector.memset(ones[:], 1.0)

            # --- gpsimd casting-load stream, priority order ---
            # rho k-tiles 0,1 -> x0 -> mu(kt0-3) -> eps0 chunk0 -> remaining
            # rho -> mu(kt4-7) -> eps0 rest -> x1, eps1, ... (the tile
            # scheduler refines by consumer deps; this sets the rough order).
            def load_rho(kt):
                nc.gpsimd.dma_start(
                    out=sig_sb[:, kt:kt + 1, :], in_=rhot_r[:, kt:kt + 1, :]
                )
                nc.scalar.activation(
                    sig_sb[:, kt, :], sig_sb[:, kt, :], ActF.Exp
                )
                nc.scalar.activation(
                    sig_sb[:, kt, :], sig_sb[:, kt, :], ActF.Ln, bias=1.0
                )

            xt_tiles = {}

            def load_x(s):
                t = xt_pool.tile([P, KT * B], FP16, tag="xt", name=f"xt_sb{s}")
                nc.gpsimd.dma_start(out=t[:], in_=xt[s])
                xt_tiles[s] = t

            load_rho(0)
            load_rho(1)
            load_x(0)
            nc.gpsimd.dma_start(out=mu_sb[:, 0:4, :], in_=mut_r[:, 0:4, :])

            eps_tiles = {}  # (s, c) -> (k0, ck, tile)

            def load_eps(s, c, k0, ck):
                t = eps_pool.tile([P, ck, OUT], FP16, tag="eps",
                                  name=f"eps_{s}_{c}")
                nc.gpsimd.dma_start(
                    out=t[:],
                    in_=epst[s, k0 * P:(k0 + ck) * P, :].rearrange(
                        "(kt p) o -> p kt o", p=P
                    ),
                )
                eps_tiles[(s, c)] = (k0, ck, t)

            load_eps(0, 0, 0, 2)
            load_rho(2)
            load_rho(3)
            load_eps(0, 1, 2, 2)
            for kt in range(4, KT):
                load_rho(kt)
            nc.gpsimd.dma_start(out=mu_sb[:, 4:KT, :], in_=mut_r[:, 4:KT, :])
            load_eps(0, 2, 4, 2)
            load_eps(0, 3, 6, 2)
            for s in range(1, SL):
                load_x(s)
                k0 = 0
                for c, ck in enumerate(CHUNKS[s]):
                    load_eps(s, c, k0, ck)
                    k0 += ck

            # bias path: softplus(brho) on ScalarE (fast [OB,P] layout),
            # combine on DVE, then SBUF->SBUF DMA into a one-partition row.
            nc.scalar.activation(sigb[:], sigb[:], ActF.Exp)
            nc.scalar.activation(sigb[:], sigb[:], ActF.Ln, bias=1.0)
            nc.vector.tensor_mul(out=bias32[:], in0=bias32[:], in1=sigb[:])
            nc.vector.tensor_add(out=bias16_2d[:], in0=bias32[:],
                                 in1=bmu_sb[:])
            # bounce through DRAM to re-lay [SL*OB, P] as one fp16 row at
            # partition 0 (rank-1 matmul weights must start at partition 0)
            nc.sync.dma_start(out=bias_scr[:], in_=bias16_2d[:])
            nc.sync.dma_start(out=bias_row[:], in_=bias_scr[:])

            # ---- main loop over local samples ----
            for s in range(SL):
                xt_sb = xt_tiles[s]
                psums = [
                    psum_pool.tile([P, 2 * B], FP32, tag=f"pb{t}",
                                   name=f"psum_{t}")
                    for t in range(OB // 2)
                ]

                for c in range(len(CHUNKS[s])):
                    k0, ck, eps_sb = eps_tiles[(s, c)]
                    for kk in range(ck):
                        kt = k0 + kk
                        # W.T = sigma.T * eps.T (+ mu.T unless on PE)
                        nc.vector.tensor_mul(
                            out=eps_sb[:, kk, :], in0=eps_sb[:, kk, :],
                            in1=sig_sb[:, kt, :],
                        )
                        if s not in MU_ON_PE:
                            nc.vector.tensor_add(
                                out=eps_sb[:, kk, :], in0=eps_sb[:, kk, :],
                                in1=mu_sb[:, kt, :],
                            )
                        rhs = xt_sb[:, kt * B:(kt + 1) * B]
                        for ob in range(OB):
                            t, j = divmod(ob, 2)
                            # start=True clears the whole psum bank's
                            # has_written flags: only the bank's first
                            # matmul carries it.
                            nc.tensor.matmul(
                                psums[t][:, j * B:(j + 1) * B],
                                eps_sb[:, kk, ob * P:(ob + 1) * P],
                                rhs,
                                start=(kt == 0 and j == 0),
                                stop=(kt == KT - 1 and s not in MU_ON_PE),
                                skip_group_check=True,
                            )
                        if s in MU_ON_PE:
                            # psum += mu.T[kt,ob].T @ X.T[kt]
                            for ob in range(OB):
                                t, j = divmod(ob, 2)
                                nc.tensor.matmul(
                                    psums[t][:, j * B:(j + 1) * B],
                                    mu_sb[:, kt, ob * P:(ob + 1) * P],
                                    rhs,
                                    start=False,
                                    stop=(kt == KT - 1),
                                    skip_group_check=True,
                                )
                        if kt == 0:
                            # rank-1 bias matmuls: psum[ob][o, b] += bias[o]
                            for ob in range(OB):
                                t, j = divmod(ob, 2)
                                so = s * OB + ob
                                nc.tensor.matmul(
                                    psums[t][:, j * B:(j + 1) * B],
                                    bias_row[0:1, so * P:(so + 1) * P],
                                    ones[:],
                                    start=False,
                                    stop=False,
                                    skip_group_check=True,
                                )

                # psum -> stage (fp16): [P,512] per copy; last sample spreads
                # copies over ScalarE/DVE/Pool so the tail drains in parallel.
                out_sb = out_pool.tile([P, OB * B], FP16)
                for t in range(OB // 2):
                    src = psums[t][:]
                    dst = out_sb[:, t * 2 * B:(t + 1) * 2 * B]
                    if s == SL - 1 and t % 2 == 1:
                        nc.vector.tensor_copy(out=dst, in_=src)
                    else:
                        nc.scalar.activation(dst, src, ActF.Identity)
                if s < SL - 1:
                    nc.scalar.dma_start(out=yt[s], in_=out_sb[:])
                else:
                    # last sample: per-bank stores so the final write is small
                    for t in range(OB // 2):
                        nc.scalar.dma_start(
                            out=yt[s][:, t * 2 * B:(t + 1) * 2 * B],
                            in_=out_sb[:, t * 2 * B:(t + 1) * 2 * B],
                        )

    nc.compile()
    return nc


def _prepare_in_maps(input, weight_mu, weight_rho, bias_mu, bias_rho, eps_w, eps_b):
    f = np.float32
    input = np.ascontiguousarray(input, dtype=f)
    eps_w = np.ascontiguousarray(eps_w, dtype=f)
    eps_b = np.asarray(eps_b, f)

    # xt[s, p, kt*B + b] = input[s, b, kt*P + p]
    xt_all = np.ascontiguousarray(
        input.reshape(S, B, KT, P).transpose(0, 3, 2, 1).reshape(S, P, KT * B)
    )
    # epst[s, i, o] = eps_w[s, o, i]
    epst_all = np.ascontiguousarray(eps_w.transpose(0, 2, 1))
    mut = np.ascontiguousarray(np.asarray(weight_mu, f).T)
    rhot = np.ascontiguousarray(np.asarray(weight_rho, f).T)
    bmu_so = np.ascontiguousarray(
        np.tile(np.asarray(bias_mu, f).reshape(OB, P), (SL, 1))
    )
    brho_so = np.ascontiguousarray(
        np.tile(np.asarray(bias_rho, f).reshape(OB, P), (SL, 1))
    )

    in_maps = []
    for c in range(NCORES):
        sl = slice(c * SL, (c + 1) * SL)
        epsb_so = np.ascontiguousarray(eps_b[sl].reshape(SL * OB, P))
        in_maps.append({
            "xt": np.ascontiguousarray(xt_all[sl]),
            "epst": np.ascontiguousarray(epst_all[sl]),
            "mut": mut,
            "rhot": rhot,
            "bmu_so": bmu_so,
            "brho_so": brho_so,
            "epsb_so": epsb_so,
        })
    return in_maps


def run(trace=False, trace_cores=None, **inputs):
    global _cached
    if _cached is None:
        _cached = build_bass()
    nc = _cached
    in_maps = _prepare_in_maps(**inputs)
    res = run_bass_kernel_spmd(
        nc,
        in_maps,
        core_ids=list(range(NCORES)),
        trace=trace,
        trace_cores=trace_cores,
    )
    # yt[s, p, ob*B+b] = out[s, b, ob*P+p] -> unpermute, upcast, gather
    outs = []
    for r in res.results:
        y = np.asarray(r["yt"], dtype=np.float32)
        y = y.reshape(SL, P, OB, B).transpose(0, 3, 2, 1).reshape(SL, B, OUT)
        outs.append(y)
    return np.ascontiguousarray(np.concatenate(outs, axis=0)), res


def kernel(**inputs) -> np.ndarray:
    out, _ = run(trace=False, **inputs)
    return out


# revision 13
# speedup vs baseline: 1.0301x; 1.0301x over previous
"""DenseVariational bass kernel for TRN2 (8 NeuronCores).

Problem: out[s,b,o] = sum_i input[s,b,i] * (mu[o,i] + softplus(rho[o,i])*eps_w[s,o,i])
                      + bias_mu[o] + softplus(bias_rho[o])*eps_b[s,o]
  S=32 samples, B=256, IN=OUT=1024, fp32 inputs.

Sharding: samples split 4-per-core across 8 cores; mu/rho replicated.

Per-core device program (mixed precision, DMA-bound design):
  - All large operands are downcast to fp16 ON INGEST via gpsimd (SWDGE)
    casting DMAs: rho/mu/x/eps stream in as fp16, halving HBM-side DMA cost.
    fp16 keeps 10 mantissa bits -> ~6e-4 relative output error, far inside
    the 2e-2 gate, while fp16 matmuls run at 1 PE cycle/row (4x fp32).
  - rho arrives in single-k-tile chunks so ScalarE softplus (Exp, Ln(1+x))
    starts ~3us in; sigma gates the first sample's DVE multiplies.
  - per sample: eps.T in 2-k-tile chunks; DVE computes W.T = sigma.T*eps.T
    (+ mu.T for samples 0-2) in place (fp16 2x mode); PE accumulates
    psum[ob] += W.T[kt,ob].T @ X.T[kt].  For the LAST sample the mu add is
    folded into PE instead (psum += mu.T[kt,ob].T @ X.T[kt]) to balance
    DVE-end against PE-end at the tail.
  - bias[s,o] enters PSUM via rank-1 matmuls (bias row stationary, ones
    moving).  The bias row is computed in a fast partition-parallel layout
    and moved to a single-partition row by a tiny SBUF->SBUF DMA.
  - PSUM -> SBUF stage (fp16) via plain [P,512] Identity copies on ScalarE;
    the last sample spreads its four copies over ScalarE/DVE/Pool so the
    tail drains in parallel; stage written to DRAM as fp16 (host upcasts).
  - PE gets NO work until ~9us; the p-state model then starts the Tensor
    engine at full clock (never-busy => fully ramped) and the stream keeps
    it busy to the end.
  - Tile pools are deep enough that loads never wait on compute.

DMA queues: gpsimd (casting loads - the serialized-bandwidth bottleneck),
sync (tiny fp32 bias loads + bias-row shuffle), scalar (fp16 stores).

Host pre-arranges layouts (pure data movement, part of sharding; no host
arithmetic - dtype casts happen on device):
  xt[s][p, kt*256+b]  = input[s, b, kt*128+p]         (fp32)
  epst[s][i, o]       = eps_w[s, o, i]                (fp32)
  mut/rhot[i, o]      = mu/rho[o, i]                  (fp32)
  epsb_so[s*8+ob, p]  = eps_b[s, ob*128+p]            (fp32)
  bmu_so/brho_so[s*8+ob, p] = bias_mu/bias_rho[ob*128+p]  (fp32, replicated)
  output yt[s][p, ob*256+b] = out[s, b, ob*128+p]     (fp16; host upcasts)
"""

import numpy as np

import concourse.bass as bass
import concourse.mybir as mybir
import concourse.tile as tile
from concourse import bacc
from concourse.bass_utils import run_bass_kernel_spmd

# Problem constants (hardcoded per harness contract)
S, B, IN, OUT = 32, 256, 1024, 1024
NCORES = 8
SL = S // NCORES          # samples per core = 4
P = 128
KT = IN // P              # 8 k-tiles
OB = OUT // P             # 8 output-row blocks
FP32 = mybir.dt.float32
FP16 = mybir.dt.float16
ActF = mybir.ActivationFunctionType

# eps chunk sizes (k-tiles) per sample; last sample tapers for a short tail
CHUNKS = [[2, 2, 2, 2], [2, 2, 2, 2], [2, 2, 2, 2], [2, 2, 2, 1, 1]]
MU_ON_PE = {1, 3}         # samples whose mu-add runs as PE matmuls

_cached = None


def build_bass():
    nc = bacc.Bacc(
        "TRN2",
        target_bir_lowering=False,
        debug=False,
        enable_asserts=False,
        num_devices=NCORES,
    )

    xt = nc.dram_tensor("xt", (SL, P, KT * B), FP32, kind="ExternalInput").ap()
    epst = nc.dram_tensor("epst", (SL, IN, OUT), FP32, kind="ExternalInput").ap()
    mut = nc.dram_tensor("mut", (IN, OUT), FP32, kind="ExternalInput").ap()
    rhot = nc.dram_tensor("rhot", (IN, OUT), FP32, kind="ExternalInput").ap()
    bmu_so = nc.dram_tensor("bmu_so", (SL * OB, P), FP32, kind="ExternalInput").ap()
    brho_so = nc.dram_tensor("brho_so", (SL * OB, P), FP32, kind="ExternalInput").ap()
    epsb_so = nc.dram_tensor("epsb_so", (SL * OB, P), FP32, kind="ExternalInput").ap()
    bias_scr = nc.dram_tensor("bias_scr", (SL * OB, P), FP16, kind="Internal").ap()
    yt = nc.dram_tensor("yt", (SL, P, OB * B), FP16, kind="ExternalOutput").ap()

    mut_r = mut.rearrange("(kt p) o -> p kt o", p=P)
    rhot_r = rhot.rearrange("(kt p) o -> p kt o", p=P)

    with tile.TileContext(nc) as tc:
        with (
            tc.tile_pool(name="persist", bufs=1) as persist,
            tc.tile_pool(name="eps", bufs=12) as eps_pool,
            tc.tile_pool(name="xtp", bufs=4) as xt_pool,
            tc.tile_pool(name="outp", bufs=2) as out_pool,
            tc.tile_pool(name="psum", bufs=2, space="PSUM") as psum_pool,
        ):
            mu_sb = persist.tile([P, KT, OUT], FP16)
            sig_sb = persist.tile([P, KT, OUT], FP16)
            sigb = persist.tile([SL * OB, P], FP32)
            bmu_sb = persist.tile([SL * OB, P], FP32)
            bias32 = persist.tile([SL * OB, P], FP32)
            bias16_2d = persist.tile([SL * OB, P], FP16)
            bias_row = persist.tile([1, SL * OB * P], FP16)
            ones = persist.tile([1, B], FP16)

            # tiny fp32 bias loads on the sync queue (HWDGE, non-cast)
            nc.sync.dma_start(out=sigb[:], in_=brho_so[:])
            nc.sync.dma_start(out=bmu_sb[:], in_=bmu_so[:])
            nc.sync.dma_start(out=bias32[:], in_=epsb_so[:])
            nc.vector.memset(ones[:], 1.0)

            # --- gpsimd casting-load stream, priority order ---
            # rho k-tiles 0,1 -> x0 -> mu(kt0-3) -> eps0 chunk0 -> remaining
            # rho -> mu(kt4-7) -> eps0 rest -> x1, eps1, ... (the tile
            # scheduler refines by consumer deps; this sets the rough order).
            def load_rho(kt):
                nc.gpsimd.dma_start(
                    out=sig_sb[:, kt:kt + 1, :], in_=rhot_r[:, kt:kt + 1, :]
                )
                nc.scalar.activation(
                    sig_sb[:, kt, :], sig_sb[:, kt, :], ActF.Exp
                )
                nc.scalar.activation(
                    sig_sb[:, kt, :], sig_sb[:, kt, :], ActF.Ln, bias=1.0
                )

            xt_tiles = {}

            def load_x(s):
                t = xt_pool.tile([P, KT * B], FP16, tag="xt", name=f"xt_sb{s}")
                nc.gpsimd.dma_start(out=t[:], in_=xt[s])
                xt_tiles[s] = t

            load_rho(0)
            load_rho(1)
            load_x(0)
            nc.gpsimd.dma_start(out=mu_sb[:, 0:4, :], in_=mut_r[:, 0:4, :])

            eps_tiles = {}  # (s, c) -> (k0, ck, tile)

            def load_eps(s, c, k0, ck):
                t = eps_pool.tile([P, ck, OUT], FP16, tag="eps",
                                  name=f"eps_{s}_{c}")
                nc.gpsimd.dma_start(
                    out=t[:],
                    in_=epst[s, k0 * P:(k0 + ck) * P, :].rearrange(
                        "(kt p) o -> p kt o", p=P
                    ),
                )
                eps_tiles[(s, c)] = (k0, ck, t)

            load_eps(0, 0, 0, 2)
            load_x(1)
            load_rho(2)
            load_rho(3)
            load_eps(0, 1, 2, 2)
            for kt in range(4, KT):
                load_rho(kt)
            nc.gpsimd.dma_start(out=mu_sb[:, 4:KT, :], in_=mut_r[:, 4:KT, :])
            load_eps(0, 2, 4, 2)
            load_eps(0, 3, 6, 2)
            for s in range(1, SL):
                if s > 1:
                    load_x(s)
                k0 = 0
                for c, ck in enumerate(CHUNKS[s]):
                    load_eps(s, c, k0, ck)
                    k0 += ck

            # bias path: softplus(brho) on ScalarE (fast [OB,P] layout),
            # combine on DVE, then SBUF->SBUF DMA into a one-partition row.
            nc.scalar.activation(sigb[:], sigb[:], ActF.Exp)
            nc.scalar.activation(sigb[:], sigb[:], ActF.Ln, bias=1.0)
            nc.vector.tensor_mul(out=bias32[:], in0=bias32[:], in1=sigb[:])
            nc.vector.tensor_add(out=bias16_2d[:], in0=bias32[:],
                                 in1=bmu_sb[:])
            # bounce through DRAM to re-lay [SL*OB, P] as one fp16 row at
            # partition 0 (rank-1 matmul weights must start at partition 0)
            nc.sync.dma_start(out=bias_scr[:], in_=bias16_2d[:])
            nc.sync.dma_start(out=bias_row[:], in_=bias_scr[:])

            # ---- main loop over local samples ----
            # psum tiles for all samples up front (slot ring 0,1,0,1); deps
            # attach at first use, so sample s+1's mu-matmuls can run as PE
            # filler inside sample s's loop once bank (s+1)%2 is released.
            all_psums = [
                [
                    psum_pool.tile([P, 2 * B], FP32, tag=f"pb{t}",
                                   name=f"psum_{si}_{t}")
                    for t in range(OB // 2)
                ]
                for si in range(SL)
            ]

            def mu_matmuls(si, kts, first):
                # psum[si] += mu.T[kt,ob].T @ X.T[kt]; only the banks' very
                # first writes (kt==0, j==0 half) carry start=True.
                psums = all_psums[si]
                for kt in kts:
                    rhs = xt_tiles[si][:, kt * B:(kt + 1) * B]
                    for ob in range(OB):
                        t, j = divmod(ob, 2)
                        nc.tensor.matmul(
                            psums[t][:, j * B:(j + 1) * B],
                            mu_sb[:, kt, ob * P:(ob + 1) * P],
                            rhs,
                            start=(first and kt == kts[0] and j == 0),
                            stop=False,
                            skip_group_check=True,
                        )

            for s in range(SL):
                xt_sb = xt_tiles[s]
                psums = all_psums[s]

                for c in range(len(CHUNKS[s])):
                    k0, ck, eps_sb = eps_tiles[(s, c)]
                    for kk in range(ck):
                        kt = k0 + kk
                        # W.T = sigma.T * eps.T (+ mu.T unless on PE)
                        nc.vector.tensor_mul(
                            out=eps_sb[:, kk, :], in0=eps_sb[:, kk, :],
                            in1=sig_sb[:, kt, :],
                        )
                        if s not in MU_ON_PE:
                            nc.vector.tensor_add(
                                out=eps_sb[:, kk, :], in0=eps_sb[:, kk, :],
                                in1=mu_sb[:, kt, :],
                            )
                        rhs = xt_sb[:, kt * B:(kt + 1) * B]
                        for ob in range(OB):
                            t, j = divmod(ob, 2)
                            # for non-MU_ON_PE samples the first eps matmul
                            # carries start=True (clears the bank's flags);
                            # MU_ON_PE samples started in their mu-matmuls.
                            nc.tensor.matmul(
                                psums[t][:, j * B:(j + 1) * B],
                                eps_sb[:, kk, ob * P:(ob + 1) * P],
                                rhs,
                                start=(kt == 0 and j == 0
                                       and s not in MU_ON_PE),
                                stop=(kt == KT - 1),
                                skip_group_check=True,
                            )
                        if kt == 0:
                            # rank-1 bias matmuls: psum[ob][o, b] += bias[o]
                            for ob in range(OB):
                                t, j = divmod(ob, 2)
                                so = s * OB + ob
                                nc.tensor.matmul(
                                    psums[t][:, j * B:(j + 1) * B],
                                    bias_row[0:1, so * P:(so + 1) * P],
                                    ones[:],
                                    start=False,
                                    stop=False,
                                    skip_group_check=True,
                                )
                    # PE filler: next sample's mu-matmuls for these k-tiles
                    if s + 1 in MU_ON_PE:
                        k0f, ckf, _ = eps_tiles[(s, c)]
                        mu_matmuls(s + 1, list(range(k0f, k0f + ckf)),
                                   first=(c == 0))

                # psum -> stage (fp16): [P,512] per copy; last sample spreads
                # copies over ScalarE/DVE/Pool so the tail drains in parallel.
                out_sb = out_pool.tile([P, OB * B], FP16)
                for t in range(OB // 2):
                    src = psums[t][:]
                    dst = out_sb[:, t * 2 * B:(t + 1) * 2 * B]
                    if s == SL - 1 and t % 2 == 1:
                        nc.vector.tensor_copy(out=dst, in_=src)
                    else:
                        nc.scalar.activation(dst, src, ActF.Identity)
                if s < SL - 1:
                    nc.scalar.dma_start(out=yt[s], in_=out_sb[:])
                else:
                    # last sample: per-bank stores so the final write is small
                    for t in range(OB // 2):
                        nc.scalar.dma_start(
                            out=yt[s][:, t * 2 * B:(t + 1) * 2 * B],
                            in_=out_sb[:, t * 2 * B:(t + 1) * 2 * B],
                        )

    nc.compile()
    return nc


def _prepare_in_maps(input, weight_mu, weight_rho, bias_mu, bias_rho, eps_w, eps_b):
    f = np.float32
    input = np.ascontiguousarray(input, dtype=f)
    eps_w = np.ascontiguousarray(eps_w, dtype=f)
    eps_b = np.asarray(eps_b, f)

    # xt[s, p, kt*B + b] = input[s, b, kt*P + p]
    xt_all = np.ascontiguousarray(
        input.reshape(S, B, KT, P).transpose(0, 3, 2, 1).reshape(S, P, KT * B)
    )
    # epst[s, i, o] = eps_w[s, o, i]
    epst_all = np.ascontiguousarray(eps_w.transpose(0, 2, 1))
    mut = np.ascontiguousarray(np.asarray(weight_mu, f).T)
    rhot = np.ascontiguousarray(np.asarray(weight_rho, f).T)
    bmu_so = np.ascontiguousarray(
        np.tile(np.asarray(bias_mu, f).reshape(OB, P), (SL, 1))
    )
    brho_so = np.ascontiguousarray(
        np.tile(np.asarray(bias_rho, f).reshape(OB, P), (SL, 1))
    )

    in_maps = []
    for c in range(NCORES):
        sl = slice(c * SL, (c + 1) * SL)
        epsb_so = np.ascontiguousarray(eps_b[sl].reshape(SL * OB, P))
        in_maps.append({
            "xt": np.ascontiguousarray(xt_all[sl]),
            "epst": np.ascontiguousarray(epst_all[sl]),
            "mut": mut,
            "rhot": rhot,
            "bmu_so": bmu_so,
            "brho_so": brho_so,
            "epsb_so": epsb_so,
        })
    return in_maps


def run(trace=False, trace_cores=None, **inputs):
    global _cached
    if _cached is None:
        _cached = build_bass()
    nc = _cached
    in_maps = _prepare_in_maps(**inputs)
    res = run_bass_kernel_spmd(
        nc,
        in_maps,
        core_ids=list(range(NCORES)),
        trace=trace,
        trace_cores=trace_cores,
    )
    # yt[s, p, ob*B+b] = out[s, b, ob*P+p] -> unpermute, upcast, gather
    outs = []
    for r in res.results:
        y = np.asarray(r["yt"], dtype=np.float32)
        y = y.reshape(SL, P, OB, B).transpose(0, 3, 2, 1).reshape(SL, B, OUT)
        outs.append(y)
    return np.ascontiguousarray(np.concatenate(outs, axis=0)), res


def kernel(**inputs) -> np.ndarray:
    out, _ = run(trace=False, **inputs)
    return out


# revision 16
# speedup vs baseline: 1.0619x; 1.0309x over previous
"""DenseVariational bass kernel for TRN2 (8 NeuronCores).

Problem: out[s,b,o] = sum_i input[s,b,i] * (mu[o,i] + softplus(rho[o,i])*eps_w[s,o,i])
                      + bias_mu[o] + softplus(bias_rho[o])*eps_b[s,o]
  S=32 samples, B=256, IN=OUT=1024, fp32 inputs.

Sharding: samples split 4-per-core across 8 cores; mu/rho replicated.

Per-core device program (mixed precision, DMA-bound design):
  - All large operands are downcast to fp16 ON INGEST via gpsimd (SWDGE)
    casting DMAs: rho/mu/x/eps stream in as fp16, halving HBM-side DMA cost.
    fp16 keeps 10 mantissa bits -> ~6e-4 relative output error, far inside
    the 2e-2 gate, while fp16 matmuls run at 1 PE cycle/row (4x fp32).
  - rho arrives in single-k-tile chunks so ScalarE softplus (Exp, Ln(1+x))
    starts ~3us in; sigma gates the first sample's DVE multiplies.
  - per sample: eps.T in 2-k-tile chunks; DVE computes W.T = sigma.T*eps.T
    (+ mu.T for samples 0-2) in place (fp16 2x mode); PE accumulates
    psum[ob] += W.T[kt,ob].T @ X.T[kt].  For the LAST sample the mu add is
    folded into PE instead (psum += mu.T[kt,ob].T @ X.T[kt]) to balance
    DVE-end against PE-end at the tail.
  - bias[s,o] enters PSUM via rank-1 matmuls (bias row stationary, ones
    moving).  The bias row is computed in a fast partition-parallel layout
    and moved to a single-partition row by a tiny SBUF->SBUF DMA.
  - PSUM -> SBUF stage (fp16) via plain [P,512] Identity copies on ScalarE;
    the last sample spreads its four copies over ScalarE/DVE/Pool so the
    tail drains in parallel; stage written to DRAM as fp16 (host upcasts).
  - PE gets NO work until ~9us; the p-state model then starts the Tensor
    engine at full clock (never-busy => fully ramped) and the stream keeps
    it busy to the end.
  - Tile pools are deep enough that loads never wait on compute.

DMA queues: gpsimd (casting loads - the serialized-bandwidth bottleneck),
sync (tiny fp32 bias loads + bias-row shuffle), scalar (fp16 stores).

Host pre-arranges layouts (pure data movement, part of sharding; no host
arithmetic - dtype casts happen on device):
  xt[s][p, kt*256+b]  = input[s, b, kt*128+p]         (fp32)
  epst[s][i, o]       = eps_w[s, o, i]                (fp32)
  mut/rhot[i, o]      = mu/rho[o, i]                  (fp32)
  epsb_so[s*8+ob, p]  = eps_b[s, ob*128+p]            (fp32)
  bmu_so/brho_so[s*8+ob, p] = bias_mu/bias_rho[ob*128+p]  (fp32, replicated)
  output yt[s][p, ob*256+b] = out[s, b, ob*128+p]     (fp16; host upcasts)
"""

import numpy as np

import concourse.bass as bass
import concourse.mybir as mybir
import concourse.tile as tile
from concourse import bacc
from concourse.bass_utils import run_bass_kernel_spmd

# Problem constants (hardcoded per harness contract)
S, B, IN, OUT = 32, 256, 1024, 1024
NCORES = 8
SL = S // NCORES          # samples per core = 4
P = 128
KT = IN // P              # 8 k-tiles
OB = OUT // P             # 8 output-row blocks
FP32 = mybir.dt.float32
FP16 = mybir.dt.float16
ActF = mybir.ActivationFunctionType

# eps chunk sizes (k-tiles) per sample; last sample tapers for a short tail
CHUNKS = [[2, 2, 2, 2], [2, 2, 2, 2], [2, 2, 2, 2], [2, 2, 2, 1, 1]]
# which (sample, kt) mu-adds run as PE matmuls (filler emitted during the
# previous sample's loop); the rest are DVE tensor-adds.  Tuned so PE-end
# and DVE-end both land just under the DMA wall.
MU_PE_KTS = {1: set(range(KT)), 3: {0, 1, 2, 3, 4}}

_cached = None


def build_bass():
    nc = bacc.Bacc(
        "TRN2",
        target_bir_lowering=False,
        debug=False,
        enable_asserts=False,
        num_devices=NCORES,
    )

    xt = nc.dram_tensor("xt", (SL, P, KT * B), FP32, kind="ExternalInput").ap()
    epst = nc.dram_tensor("epst", (SL, IN, OUT), FP32, kind="ExternalInput").ap()
    mut = nc.dram_tensor("mut", (IN, OUT), FP32, kind="ExternalInput").ap()
    rhot = nc.dram_tensor("rhot", (IN, OUT), FP32, kind="ExternalInput").ap()
    bmu_so = nc.dram_tensor("bmu_so", (SL * OB, P), FP32, kind="ExternalInput").ap()
    brho_so = nc.dram_tensor("brho_so", (SL * OB, P), FP32, kind="ExternalInput").ap()
    epsb_so = nc.dram_tensor("epsb_so", (SL * OB, P), FP32, kind="ExternalInput").ap()
    bias_scr = nc.dram_tensor("bias_scr", (SL * OB, P), FP16, kind="Internal").ap()
    yt = nc.dram_tensor("yt", (SL, P, OB * B), FP16, kind="ExternalOutput").ap()

    mut_r = mut.rearrange("(kt p) o -> p kt o", p=P)
    rhot_r = rhot.rearrange("(kt p) o -> p kt o", p=P)

    with tile.TileContext(nc) as tc:
        with (
            tc.tile_pool(name="persist", bufs=1) as persist,
            tc.tile_pool(name="eps", bufs=12) as eps_pool,
            tc.tile_pool(name="xtp", bufs=4) as xt_pool,
            tc.tile_pool(name="outp", bufs=2) as out_pool,
            tc.tile_pool(name="psum", bufs=2, space="PSUM") as psum_pool,
        ):
            mu_sb = persist.tile([P, KT, OUT], FP16)
            sig_sb = persist.tile([P, KT, OUT], FP16)
            sigb = persist.tile([SL * OB, P], FP32)
            bmu_sb = persist.tile([SL * OB, P], FP32)
            bias32 = persist.tile([SL * OB, P], FP32)
            bias16_2d = persist.tile([SL * OB, P], FP16)
            bias_row = persist.tile([1, SL * OB * P], FP16)
            ones = persist.tile([1, B], FP16)
            zeros_row = persist.tile([1, B], FP16)

            # tiny fp32 bias loads on the sync queue (HWDGE, non-cast)
            nc.sync.dma_start(out=sigb[:], in_=brho_so[:])
            nc.sync.dma_start(out=bmu_sb[:], in_=bmu_so[:])
            nc.sync.dma_start(out=bias32[:], in_=epsb_so[:])
            nc.vector.memset(ones[:], 1.0)
            nc.vector.memset(zeros_row[:], 0.0)

            # --- gpsimd casting-load stream, priority order ---
            # rho k-tiles 0,1 -> x0 -> mu(kt0-3) -> eps0 chunk0 -> remaining
            # rho -> mu(kt4-7) -> eps0 rest -> x1, eps1, ... (the tile
            # scheduler refines by consumer deps; this sets the rough order).
            def load_rho(kt):
                nc.gpsimd.dma_start(
                    out=sig_sb[:, kt:kt + 1, :], in_=rhot_r[:, kt:kt + 1, :]
                )
                nc.scalar.activation(
                    sig_sb[:, kt, :], sig_sb[:, kt, :], ActF.Exp
                )
                nc.scalar.activation(
                    sig_sb[:, kt, :], sig_sb[:, kt, :], ActF.Ln, bias=1.0
                )

            xt_tiles = {}

            def load_x(s):
                t = xt_pool.tile([P, KT * B], FP16, tag="xt", name=f"xt_sb{s}")
                nc.gpsimd.dma_start(out=t[:], in_=xt[s])
                xt_tiles[s] = t

            load_rho(0)
            load_rho(1)
            load_x(0)
            nc.gpsimd.dma_start(out=mu_sb[:, 0:4, :], in_=mut_r[:, 0:4, :])

            eps_tiles = {}  # (s, c) -> (k0, ck, tile)

            def load_eps(s, c, k0, ck):
                t = eps_pool.tile([P, ck, OUT], FP16, tag="eps",
                                  name=f"eps_{s}_{c}")
                nc.gpsimd.dma_start(
                    out=t[:],
                    in_=epst[s, k0 * P:(k0 + ck) * P, :].rearrange(
                        "(kt p) o -> p kt o", p=P
                    ),
                )
                eps_tiles[(s, c)] = (k0, ck, t)

            load_eps(0, 0, 0, 2)
            load_x(1)
            load_rho(2)
            load_rho(3)
            load_eps(0, 1, 2, 2)
            for kt in range(4, KT):
                load_rho(kt)
            nc.gpsimd.dma_start(out=mu_sb[:, 4:KT, :], in_=mut_r[:, 4:KT, :])
            load_eps(0, 2, 4, 2)
            load_eps(0, 3, 6, 2)
            for s in range(1, SL):
                if s > 1:
                    load_x(s)
                k0 = 0
                for c, ck in enumerate(CHUNKS[s]):
                    load_eps(s, c, k0, ck)
                    k0 += ck

            # bias path: softplus(brho) on ScalarE (fast [OB,P] layout),
            # combine on DVE, then SBUF->SBUF DMA into a one-partition row.
            nc.scalar.activation(sigb[:], sigb[:], ActF.Exp)
            nc.scalar.activation(sigb[:], sigb[:], ActF.Ln, bias=1.0)
            nc.vector.tensor_mul(out=bias32[:], in0=bias32[:], in1=sigb[:])
            nc.vector.tensor_add(out=bias16_2d[:], in0=bias32[:],
                                 in1=bmu_sb[:])
            # bounce through DRAM to re-lay [SL*OB, P] as one fp16 row at
            # partition 0 (rank-1 matmul weights must start at partition 0)
            nc.sync.dma_start(out=bias_scr[:], in_=bias16_2d[:])
            nc.sync.dma_start(out=bias_row[:], in_=bias_scr[:])

            # ---- main loop over local samples ----
            # psum tiles for all samples up front (slot ring 0,1,0,1); deps
            # attach at first use, so sample s+1's mu-matmuls can run as PE
            # filler inside sample s's loop once bank (s+1)%2 is released.
            all_psums = [
                [
                    psum_pool.tile([P, 2 * B], FP32, tag=f"pb{t}",
                                   name=f"psum_{si}_{t}")
                    for t in range(OB // 2)
                ]
                for si in range(SL)
            ]

            def mu_matmuls(si, kts, first):
                # psum[si] += mu.T[kt,ob].T @ X.T[kt].
                #
                # If bank-set si%2 had a previous tenant (sample si-2), its
                # stage copies (ScalarE/DVE readers) must finish before these
                # PE writes; PE program order alone does not enforce that
                # cross-engine WAR edge.  Guard matmuls make it explicit:
                # each reads one element of sample si-2's stage tile (so it
                # waits on that bank's copy), multiplies by a zeros row, and
                # carries start=True to clear the bank's has_written flags.
                psums = all_psums[si]
                if first:
                    if si >= 2:
                        stage_prev = out_tiles[si - 2]
                        for t in range(OB // 2):
                            nc.tensor.matmul(
                                psums[t][:, 0:B],
                                stage_prev[0:1, t * 2 * B:t * 2 * B + P],
                                zeros_row[:],
                                start=True,
                                stop=False,
                                skip_group_check=True,
                            )
                        first = False
                for kt in kts:
                    rhs = xt_tiles[si][:, kt * B:(kt + 1) * B]
                    for ob in range(OB):
                        t, j = divmod(ob, 2)
                        nc.tensor.matmul(
                            psums[t][:, j * B:(j + 1) * B],
                            mu_sb[:, kt, ob * P:(ob + 1) * P],
                            rhs,
                            start=(first and kt == kts[0] and j == 0),
                            stop=False,
                            skip_group_check=True,
                        )

            out_tiles = {}
            for s in range(SL):
                xt_sb = xt_tiles[s]
                psums = all_psums[s]

                for c in range(len(CHUNKS[s])):
                    k0, ck, eps_sb = eps_tiles[(s, c)]
                    for kk in range(ck):
                        kt = k0 + kk
                        # W.T = sigma.T * eps.T (+ mu.T unless on PE)
                        nc.vector.tensor_mul(
                            out=eps_sb[:, kk, :], in0=eps_sb[:, kk, :],
                            in1=sig_sb[:, kt, :],
                        )
                        if kt not in MU_PE_KTS.get(s, ()):
                            nc.vector.tensor_add(
                                out=eps_sb[:, kk, :], in0=eps_sb[:, kk, :],
                                in1=mu_sb[:, kt, :],
                            )
                        rhs = xt_sb[:, kt * B:(kt + 1) * B]
                        for ob in range(OB):
                            t, j = divmod(ob, 2)
                            # for non-MU_ON_PE samples the first eps matmul
                            # carries start=True (clears the bank's flags);
                            # MU_ON_PE samples started in their mu-matmuls.
                            nc.tensor.matmul(
                                psums[t][:, j * B:(j + 1) * B],
                                eps_sb[:, kk, ob * P:(ob + 1) * P],
                                rhs,
                                start=(kt == 0 and j == 0
                                       and s not in MU_PE_KTS),
                                stop=(kt == KT - 1),
                                skip_group_check=True,
                            )
                        if kt == 0:
                            # rank-1 bias matmuls: psum[ob][o, b] += bias[o]
                            for ob in range(OB):
                                t, j = divmod(ob, 2)
                                so = s * OB + ob
                                nc.tensor.matmul(
                                    psums[t][:, j * B:(j + 1) * B],
                                    bias_row[0:1, so * P:(so + 1) * P],
                                    ones[:],
                                    start=False,
                                    stop=False,
                                    skip_group_check=True,
                                )
                    # PE filler: next sample's mu-matmuls for these k-tiles
                    if s + 1 in MU_PE_KTS:
                        k0f, ckf, _ = eps_tiles[(s, c)]
                        kts = [kt for kt in range(k0f, k0f + ckf)
                               if kt in MU_PE_KTS[s + 1]]
                        if kts:
                            mu_matmuls(s + 1, kts, first=(c == 0))

                # psum -> stage (fp16): [P,512] per copy; last sample spreads
                # copies over ScalarE/DVE/Pool so the tail drains in parallel.
                out_sb = out_pool.tile([P, OB * B], FP16, tag="out",
                                       name=f"out_sb{s}")
                out_tiles[s] = out_sb
                for t in range(OB // 2):
                    src = psums[t][:]
                    dst = out_sb[:, t * 2 * B:(t + 1) * 2 * B]
                    if s == SL - 1 and t % 2 == 1:
                        nc.vector.tensor_copy(out=dst, in_=src)
                    else:
                        nc.scalar.activation(dst, src, ActF.Identity)
                if s < SL - 1:
                    nc.scalar.dma_start(out=yt[s], in_=out_sb[:])
                else:
                    # last sample: per-bank stores so the final write is small
                    for t in range(OB // 2):
                        nc.scalar.dma_start(
                            out=yt[s][:, t * 2 * B:(t + 1) * 2 * B],
                            in_=out_sb[:, t * 2 * B:(t + 1) * 2 * B],
                        )

    nc.compile()
    return nc


def _prepare_in_maps(input, weight_mu, weight_rho, bias_mu, bias_rho, eps_w, eps_b):
    f = np.float32
    input = np.ascontiguousarray(input, dtype=f)
    eps_w = np.ascontiguousarray(eps_w, dtype=f)
    eps_b = np.asarray(eps_b, f)

    # xt[s, p, kt*B + b] = input[s, b, kt*P + p]
    xt_all = np.ascontiguousarray(
        input.reshape(S, B, KT, P).transpose(0, 3, 2, 1).reshape(S, P, KT * B)
    )
    # epst[s, i, o] = eps_w[s, o, i]
    epst_all = np.ascontiguousarray(eps_w.transpose(0, 2, 1))
    mut = np.ascontiguousarray(np.asarray(weight_mu, f).T)
    rhot = np.ascontiguousarray(np.asarray(weight_rho, f).T)
    bmu_so = np.ascontiguousarray(
        np.tile(np.asarray(bias_mu, f).reshape(OB, P), (SL, 1))
    )
    brho_so = np.ascontiguousarray(
        np.tile(np.asarray(bias_rho, f).reshape(OB, P), (SL, 1))
    )

    in_maps = []
    for c in range(NCORES):
        sl = slice(c * SL, (c + 1) * SL)
        epsb_so = np.ascontiguousarray(eps_b[sl].reshape(SL * OB, P))
        in_maps.append({
            "xt": np.ascontiguousarray(xt_all[sl]),
            "epst": np.ascontiguousarray(epst_all[sl]),
            "mut": mut,
            "rhot": rhot,
            "bmu_so": bmu_so,
            "brho_so": brho_so,
            "epsb_so": epsb_so,
        })
    return in_maps


def run(trace=False, trace_cores=None, **inputs):
    global _cached
    if _cached is None:
        _cached = build_bass()
    nc = _cached
    in_maps = _prepare_in_maps(**inputs)
    res = run_bass_kernel_spmd(
        nc,
        in_maps,
        core_ids=list(range(NCORES)),
        trace=trace,
        trace_cores=trace_cores,
    )
    # yt[s, p, ob*B+b] = out[s, b, ob*P+p] -> unpermute, upcast, gather
    outs = []
    for r in res.results:
        y = np.asarray(r["yt"], dtype=np.float32)
        y = y.reshape(SL, P, OB, B).transpose(0, 3, 2, 1).reshape(SL, B, OUT)
        outs.append(y)
    return np.ascontiguousarray(np.concatenate(outs, axis=0)), res


def kernel(**inputs) -> np.ndarray:
    out, _ = run(trace=False, **inputs)
    return out
